# revision 8
# baseline (speedup 1.0000x reference)
"""Trainium2 Bass kernel for nn_Conv_MS_MSA (spectral multi-head self-attention).

Reference computation (per batch):
  qkv = dw3x3_depthwise(conv1x1(x))          # 256 -> 768 ch, then per-ch 3x3
  q, k, v = split(qkv); v_out = v
  per head (8 heads x 32 d): L2-normalize q,k rows over the 65536 pixels,
  attn = softmax(k_norm @ q_norm^T * rescale), out = attn @ v
  out_c = conv3x3_dense(out, w_proj)         # 256 -> 256 ch

Sharding: spatial bands. Core i owns image rows [32i, 32i+32) of BOTH batches,
with halo rows for the two 3x3 convs. The only global coupling is the per-head
32x32 Gram matrices and q/k row norms -- tiny sums over pixels -- reduced with
one ~70KB on-device AllReduce mid-kernel. Everything else is band-local.

Layouts on device: channels on SBUF partitions, pixels on the free dim.
Matmuls run in float32r (full PE rate at free-dim>=256). The depthwise conv is
split tap-wise across PE (diagonal matmuls), DVE and GPSIMD (MAC chains).
"""

import sys

if "/opt/trn_rl_repo" not in sys.path:
    sys.path.insert(0, "/opt/trn_rl_repo")

import numpy as np

import concourse.bass as bass
import concourse.tile as tile
from concourse import bacc, mybir
from concourse import bass_utils

# ---------------------------------------------------------------- problem dims
B = 2
C = 256
H = 256
W = 256
HEADS = 8
N_CORES = 8
ROWS = H // N_CORES          # 32 owned rows per core
VB = ROWS + 2                # 34 v/out band rows (1-row halo each side)
XB = ROWS + 4                # 36 x/qkv band rows (2-row halo each side)
CT = C // 128                # 2 channel tiles of 128 per 256-ch tensor
QKCT = 4                     # q,k channel tiles (512 ch)
EPS = 1e-12

fp32 = mybir.dt.float32
fp32r = mybir.dt.float32r

# tap engine assignment (tap = dy*3+dx in [0,9)); tunables
QK_PE_TAPS = [0, 1, 2]
QK_GP_TAPS = []
V_PE_TAPS = [0]
V_GP_TAPS = []

CHUNK = 6                    # rows per processing chunk

Alu = mybir.AluOpType
Act = mybir.ActivationFunctionType


_CONST_POOL = None


def _single(tc, shape, dtype, name):
    t = _CONST_POOL.tile(shape, dtype, tag=name, name=name)
    return t


def _chunks(total, step):
    out = []
    s = 0
    while s < total:
        out.append((s, min(step, total - s)))
        s += step
    return out


def _dw_chain(nc, qk3, qkv_t, wdw_sb, t, pe_taps, gp_taps, L):
    """Depthwise-conv MAC chain for non-PE taps, into qk3 ([128, L, 256]).
    PE taps (if any) were already evacuated into qk3; otherwise the first
    op initializes it."""
    first = len(pe_taps) == 0
    dve_taps = [tp for tp in range(9) if tp not in pe_taps and tp not in gp_taps]
    for eng, taps in ((nc.gpsimd, gp_taps), (nc.vector, dve_taps)):
        for tp in taps:
            dy, dx = tp // 3, tp % 3
            win = qkv_t[:, dy : dy + L, dx : dx + 256]
            sc = wdw_sb[:, t, tp : tp + 1]
            if first:
                eng.tensor_scalar_mul(qk3, win, sc)
                first = False
            else:
                eng.scalar_tensor_tensor(
                    qk3, win, sc, qk3, op0=Alu.mult, op1=Alu.add
                )


def _dw_pe_taps(nc, ps_pool, qk3, qkv_t, diags, t, pe_taps, L):
    """PE-tap part of the depthwise conv: diagonal matmuls accumulated in
    PSUM, ACT-evacuated into qk3 ([128, L, 256])."""
    if not pe_taps:
        return
    for n in range((L * 256) // 512):
        ps = ps_pool.tile([128, 2, 256], fp32, tag="psdw")
        for j, tp in enumerate(pe_taps):
            dy, dx = tp // 3, tp % 3
            rhs = qkv_t[:, 2 * n + dy : 2 * n + dy + 2, dx : dx + 256]
            nc.tensor.matmul(
                ps[:],
                diags[(t, tp)][:],
                rhs,
                start=(j == 0),
                stop=(j == len(pe_taps) - 1),
            )
        nc.scalar.copy(qk3[:, 2 * n : 2 * n + 2, :], ps[:])


def build_program():
    nc = bacc.Bacc(
        "TRN2", target_bir_lowering=False, debug=False, num_devices=N_CORES
    )

    # ------------------------------------------------------------- DRAM I/O
    x_d = nc.dram_tensor("x", [B, CT, 128, XB, 256], fp32r, kind="ExternalInput")
    wq_d = nc.dram_tensor("wq", [128, CT, 768], fp32r, kind="ExternalInput")
    wdw_d = nc.dram_tensor("wdw", [128, 6, 9], fp32, kind="ExternalInput")
    wp_d = nc.dram_tensor("wp", [128, CT, 9, 256], fp32r, kind="ExternalInput")
    ident_d = nc.dram_tensor("ident", [128, 128], fp32r, kind="ExternalInput")
    resc_d = nc.dram_tensor("resc", [128, CT], fp32, kind="ExternalInput")
    hmask_d = nc.dram_tensor("hmask", [128, 2], fp32, kind="ExternalInput")

    vband_d = nc.dram_tensor(
        "vband", [B, CT, 128, VB, 256], fp32, kind="ExternalOutput"
    )
    outc_d = nc.dram_tensor(
        "outc", [B, CT, 128, ROWS, 256], fp32, kind="ExternalOutput"
    )

    with tile.TileContext(nc) as tc:
        global _CONST_POOL
        with tc.tile_pool(name="consts", bufs=1) as cpool:
            _CONST_POOL = cpool
            _build(nc, tc, x_d, wq_d, wdw_d, wp_d, ident_d, resc_d, hmask_d,
                   vband_d, outc_d)
            _CONST_POOL = None
    nc.compile()
    return nc


def _build(nc, tc, x_d, wq_d, wdw_d, wp_d, ident_d, resc_d, hmask_d,
           vband_d, outc_d):
    # ------------------------------------------------------ constants in SBUF
    wq = _single(tc, [128, CT, 768], fp32r, name="wq_sb")
    wdw = _single(tc, [128, 6, 9], fp32, name="wdw_sb")
    wp = _single(tc, [128, CT, 9, 256], fp32r, name="wp_sb")
    ident = _single(tc, [128, 128], fp32r, name="ident_sb")
    resc = _single(tc, [128, CT], fp32, name="resc_sb")
    hmask = _single(tc, [128, 2], fp32, name="hmask_sb")
    nc.sync.dma_start(wq[:], wq_d[:, :, :])
    nc.sync.dma_start(wdw[:], wdw_d[:, :, :])
    nc.sync.dma_start(wp[:], wp_d[:, :, :, :])
    nc.sync.dma_start(ident[:], ident_d[:, :])
    nc.sync.dma_start(resc[:], resc_d[:, :])
    nc.sync.dma_start(hmask[:], hmask_d[:, :])

    # diagonal weight matrices for PE depthwise taps
    diags = {}
    diag_specs = [(t, tp) for t in range(QKCT) for tp in QK_PE_TAPS] + [
        (QKCT + t, tp) for t in range(CT) for tp in V_PE_TAPS
    ]
    for t, tp in diag_specs:
        d = _single(tc, [128, 128], fp32r, name=f"diag_{t}_{tp}")
        nc.vector.tensor_scalar_mul(d[:], ident[:], wdw[:, t, tp : tp + 1])
        diags[(t, tp)] = d

    # global accumulators
    stats = _single(tc, [128, 136], fp32, name="stats_sb")
    gacc = _single(tc, [128, B, 2, 256], fp32, name="gacc_sb")
    nc.gpsimd.memset(stats[:], 0.0)
    nc.gpsimd.memset(gacc[:], 0.0)

    # =========================================================== QK pass
    # owned v-band rows [1, 33): q,k, their sumsq, and the raw Gram.
    with (
        tc.tile_pool(name="xband", bufs=1) as p_x,
        tc.tile_pool(name="qkvt", bufs=5) as p_qkv,
        tc.tile_pool(name="qkp", bufs=5) as p_qk,
        tc.tile_pool(name="sqp", bufs=3) as p_sq,
        tc.tile_pool(name="scrp", bufs=2) as p_scr,
        tc.tile_pool(name="qtp", bufs=2) as p_qt,
        tc.tile_pool(name="psc", bufs=2, space="PSUM") as ps_conv,
        tc.tile_pool(name="psd", bufs=2, space="PSUM") as ps_dw,
        tc.tile_pool(name="pst", bufs=2, space="PSUM") as ps_tr,
        tc.tile_pool(name="psg", bufs=2, space="PSUM") as ps_gram,
    ):
        for b in range(B):
            x_sb = p_x.tile([128, CT, XB, 256], fp32r, tag="x")
            for kt in range(CT):
                nc.sync.dma_start(x_sb[:, kt], x_d[b, kt])

            for s, L in _chunks(ROWS, CHUNK):
                s += 1  # band rows [1, 33)
                qk_tiles = []
                for t in range(QKCT):
                    qkv_t = p_qkv.tile([128, L + 2, 258], fp32r, tag="qkvt")
                    nc.gpsimd.memset(qkv_t[:, :, 0].bitcast(fp32), 0.0)
                    nc.gpsimd.memset(qkv_t[:, :, 257].bitcast(fp32), 0.0)
                    for n in range((L + 2) // 2):
                        ps = ps_conv.tile([128, 2, 256], fp32, tag="psc")
                        for kt in range(CT):
                            rhs = x_sb[:, kt, s + 2 * n : s + 2 * n + 2, :]
                            nc.tensor.matmul(
                                ps[:],
                                wq[:, kt, t * 128 : (t + 1) * 128],
                                rhs,
                                start=(kt == 0),
                                stop=(kt == CT - 1),
                            )
                        nc.scalar.copy(qkv_t[:, 2 * n : 2 * n + 2, 1:257], ps[:])
                    qk_t = p_qk.tile([128, L, 256], fp32, tag="qk")
                    qk3 = qk_t[:, :, :]
                    _dw_pe_taps(nc, ps_dw, qk3, qkv_t, diags, t, QK_PE_TAPS, L)
                    _dw_chain(nc, qk3, qkv_t, wdw, t, QK_PE_TAPS, QK_GP_TAPS, L)
                    qk_tiles.append(qk_t)

                    # sumsq of this chunk -> stats col 128 + b*4 + t
                    scr = p_scr.tile([128, L, 256], fp32, tag="scr")
                    sq = p_sq.tile([128, 1], fp32, tag="sq")
                    nc.scalar.activation(
                        scr[:], qk_t[:], Act.Square, accum_out=sq[:]
                    )
                    col = 128 + b * 4 + t
                    nc.vector.tensor_tensor(
                        stats[:, col : col + 1],
                        stats[:, col : col + 1],
                        sq[:],
                        op=Alu.add,
                    )

                # transposes + Gram over this chunk's pixels
                nblk = (L * 256) // 128
                g_ps = [
                    ps_gram.tile([128, 256], fp32, tag="psg", name="gps") for _ in range(2)
                ]
                for blk in range(nblk):
                    r, cb = blk // 2, (blk % 2) * 128
                    qt_t = p_qt.tile([128, 256], fp32r, tag="qt")
                    kt_t = p_qt.tile([128, 256], fp32r, tag="kt")
                    for half in range(2):
                        src_q = qk_tiles[half][:, r, cb : cb + 128]
                        src_k = qk_tiles[2 + half][:, r, cb : cb + 128]
                        ps_q = ps_tr.tile([128, 128], fp32, tag="pst")
                        ps_k = ps_tr.tile([128, 128], fp32, tag="pst")
                        nc.tensor.transpose(
                            ps_q[:], src_q, ident[:].bitcast(fp32)
                        )
                        nc.tensor.transpose(
                            ps_k[:], src_k, ident[:].bitcast(fp32)
                        )
                        nc.scalar.copy(
                            qt_t[:, half * 128 : half * 128 + 128], ps_q[:]
                        )
                        nc.scalar.copy(
                            kt_t[:, half * 128 : half * 128 + 128], ps_k[:]
                        )
                    for g in range(2):
                        nc.tensor.matmul(
                            g_ps[g][:],
                            kt_t[:, g * 128 : (g + 1) * 128],
                            qt_t[:],
                            start=(blk == 0),
                            stop=(blk == nblk - 1),
                            skip_group_check=True,
                        )
                for g in range(2):
                    nc.vector.tensor_tensor(
                        gacc[:, b, g, :], gacc[:, b, g, :], g_ps[g][:],
                        op=Alu.add,
                    )

    # extract per-head diagonal 32x32 blocks of the Gram into stats cols
    for b in range(B):
        for g in range(2):
            for i in range(4):
                h = 4 * g + i
                src = gacc[32 * i : 32 * i + 32, b, g, 32 * h : 32 * h + 32]
                dst = stats[32 * i : 32 * i + 32, (2 * b + g) * 32 :][:, :32]
                nc.vector.tensor_copy(dst, src)

    # ============================================================ AllReduce
    with tc.tile_pool(name="ardram", bufs=1, space="DRAM") as p_ar:
        ar_in = p_ar.tile([128, 136], fp32)
        ar_out = p_ar.tile([128, 136], fp32, addr_space="Shared")
        nc.sync.dma_start(ar_in[:], stats[:])
        nc.gpsimd.collective_compute(
            "AllReduce",
            Alu.add,
            replica_groups=[list(range(N_CORES))],
            ins=[ar_in[:].opt()],
            outs=[ar_out[:].opt()],
        )
        stats2 = _single(tc, [128, 136], fp32, name="stats2_sb")
        nc.sync.dma_start(stats2[:], ar_out[:])

    # ====================================================== softmax -> attnT
    # rsq[:, idx] = 1 / max(sqrt(sumsq), eps), idx = b*4 + qk*2 + g
    rsq = _single(tc, [128, 8], fp32, name="rsq_sb")
    nc.scalar.activation(rsq[:], stats2[:, 128:136], Act.Sqrt)
    nc.vector.tensor_scalar_max(rsq[:], rsq[:], EPS)
    nc.vector.reciprocal(rsq[:], rsq[:])

    bd = {}
    with tc.tile_pool(name="smx", bufs=4) as p_sm:
        for b in range(B):
            for g in range(2):
                kcol = b * 4 + 2 + g
                qcol = b * 4 + g
                ksc = p_sm.tile([128, 1], fp32, tag="ksc")
                nc.vector.tensor_tensor(
                    ksc[:], rsq[:, kcol : kcol + 1], resc[:, g : g + 1],
                    op=Alu.mult,
                )
                t1 = p_sm.tile([128, 32], fp32, tag="t1")
                graw = stats2[:, (2 * b + g) * 32 :][:, :32]
                nc.vector.tensor_scalar_mul(t1[:], graw, ksc[:])
                # M[p, j] = rsq_q[32*(p//32) + j]: broadcast + block-transpose
                a2 = p_sm.tile([128, 32], fp32, tag="a2")
                nc.vector.tensor_scalar(
                    a2[:], t1[:], 0.0, rsq[:, qcol : qcol + 1],
                    op0=Alu.mult, op1=Alu.add,
                )
                m = p_sm.tile([128, 32], fp32, tag="m")
                nc.vector.transpose(m[:], a2[:])
                nc.vector.tensor_tensor(t1[:], t1[:], m[:], op=Alu.mult)
                # softmax over the free (e) dim
                mx = p_sm.tile([128, 1], fp32, tag="mx")
                nc.vector.tensor_reduce(
                    mx[:], t1[:], mybir.AxisListType.X, Alu.max
                )
                nc.vector.tensor_scalar_sub(t1[:], t1[:], mx[:])
                ex = p_sm.tile([128, 32], fp32, tag="ex")
                nc.scalar.activation(ex[:], t1[:], Act.Exp)
                sm = p_sm.tile([128, 1], fp32, tag="sm")
                nc.vector.tensor_reduce(
                    sm[:], ex[:], mybir.AxisListType.X, Alu.add
                )
                nc.vector.reciprocal(sm[:], sm[:])
                at = p_sm.tile([128, 32], fp32, tag="at")
                nc.vector.tensor_scalar_mul(at[:], ex[:], sm[:])
                att = p_sm.tile([128, 32], fp32, tag="att")
                nc.vector.transpose(att[:], at[:])
                # block-diagonal lhsT for the attn@v matmul
                bdt = _single(tc, [128, 128], fp32r, name=f"bd_{b}_{g}")
                nc.gpsimd.memset(bdt[:].bitcast(fp32), 0.0)
                for i in range(4):
                    nc.vector.tensor_copy(
                        bdt[32 * i : 32 * i + 32, 32 * i : 32 * i + 32],
                        att[32 * i : 32 * i + 32, :],
                    )
                bd[(b, g)] = bdt

    # ================================================= V + attn + proj pass
    with (
        tc.tile_pool(name="xc2", bufs=2) as p_x2,
        tc.tile_pool(name="qkvt2", bufs=3) as p_qkv2,
        tc.tile_pool(name="vtp", bufs=3) as p_v,
        tc.tile_pool(name="vrp", bufs=3) as p_vr,
        tc.tile_pool(name="outb", bufs=2) as p_out,
        tc.tile_pool(name="ocp", bufs=3) as p_oc,
        tc.tile_pool(name="psc2", bufs=2, space="PSUM") as ps_conv2,
        tc.tile_pool(name="psd2", bufs=2, space="PSUM") as ps_dw2,
        tc.tile_pool(name="psa", bufs=2, space="PSUM") as ps_attn,
        tc.tile_pool(name="psp", bufs=2, space="PSUM") as ps_proj,
    ):
        for b in range(B):
            for m0, L in _chunks(ROWS, CHUNK):
                LV = L + 2            # v/out band rows in this chunk
                LX = L + 4            # x/qkv rows in this chunk
                x_c = p_x2.tile([128, CT, LX, 256], fp32r, tag="xc")
                for kt in range(CT):
                    nc.sync.dma_start(
                        x_c[:, kt], x_d[b, kt][:, m0 : m0 + LX, :]
                    )

                v_tiles = []
                for t in range(CT):
                    qkv_t = p_qkv2.tile([128, LX, 258], fp32r, tag="qkvt2")
                    nc.gpsimd.memset(qkv_t[:, :, 0].bitcast(fp32), 0.0)
                    nc.gpsimd.memset(qkv_t[:, :, 257].bitcast(fp32), 0.0)
                    for n in range(LX // 2):
                        ps = ps_conv2.tile([128, 2, 256], fp32, tag="psc2")
                        for kt in range(CT):
                            rhs = x_c[:, kt, 2 * n : 2 * n + 2, :]
                            nc.tensor.matmul(
                                ps[:],
                                wq[:, kt, (QKCT + t) * 128 :][:, :128],
                                rhs,
                                start=(kt == 0),
                                stop=(kt == CT - 1),
                            )
                        nc.scalar.copy(qkv_t[:, 2 * n : 2 * n + 2, 1:257], ps[:])
                    v_t = p_v.tile([128, LV, 256], fp32, tag="vt")
                    v3 = v_t[:, :, :]
                    _dw_pe_taps(
                        nc, ps_dw2, v3, qkv_t, diags, QKCT + t, V_PE_TAPS, LV
                    )
                    _dw_chain(
                        nc, v3, qkv_t, wdw, QKCT + t, V_PE_TAPS, V_GP_TAPS, LV
                    )
                    # halo masking at image edges
                    if m0 == 0:
                        nc.vector.tensor_scalar_mul(
                            v_t[:, 0, :], v_t[:, 0, :], hmask[:, 0:1]
                        )
                    if m0 + LV == VB:
                        nc.vector.tensor_scalar_mul(
                            v_t[:, LV - 1, :], v_t[:, LV - 1, :], hmask[:, 1:2]
                        )
                    v_tiles.append(v_t)
                    # vband rows [m0, m0+CHUNK), all remaining on last chunk
                    wrows = LV if m0 + LV == VB else L
                    nc.sync.dma_start(
                        vband_d[b, t][:, m0 : m0 + wrows, :],
                        v_t[:, 0:wrows, :],
                    )

                # out = attn @ v  (block-diag lhsT per channel-tile)
                out_tiles = []
                for g in range(CT):
                    v_r = p_vr.tile([128, LV, 256], fp32r, tag="vr")
                    nc.scalar.copy(v_r[:], v_tiles[g][:])
                    o_t = p_out.tile([128, LV, 258], fp32r, tag="outb")
                    nc.gpsimd.memset(o_t[:, :, 0].bitcast(fp32), 0.0)
                    nc.gpsimd.memset(o_t[:, :, 257].bitcast(fp32), 0.0)
                    for n in range(LV // 2):
                        ps = ps_attn.tile([128, 2, 256], fp32, tag="psa")
                        nc.tensor.matmul(
                            ps[:],
                            bd[(b, g)][:],
                            v_r[:, 2 * n : 2 * n + 2, :],
                            start=True,
                            stop=True,
                        )
                        nc.scalar.copy(o_t[:, 2 * n : 2 * n + 2, 1:257], ps[:])
                    out_tiles.append(o_t)

                # proj 3x3 over out band rows -> out_c rows [m0, m0+L)
                for mt in range(CT):
                    oc_t = p_oc.tile([128, L, 256], fp32, tag="oc")
                    for n in range(L // 2):
                        ps = ps_proj.tile([128, 2, 256], fp32, tag="psp")
                        idx = 0
                        for tp in range(9):
                            dy, dx = tp // 3, tp % 3
                            for kt in range(CT):
                                rhs = out_tiles[kt][
                                    :, 2 * n + dy : 2 * n + dy + 2,
                                    dx : dx + 256,
                                ]
                                nc.tensor.matmul(
                                    ps[:],
                                    wp[:, kt, tp, mt * 128 :][:, :128],
                                    rhs,
                                    start=(idx == 0),
                                    stop=(idx == 17),
                                    skip_group_check=True,
                                )
                                idx += 1
                        nc.scalar.copy(oc_t[:, 2 * n : 2 * n + 2, :], ps[:])
                    nc.sync.dma_start(
                        outc_d[b, mt][:, m0 : m0 + L, :], oc_t[:]
                    )


# ------------------------------------------------------------------- host side
_NC_CACHE = None


def _get_program():
    global _NC_CACHE
    if _NC_CACHE is None:
        _NC_CACHE = build_program()
    return _NC_CACHE


def kernel(x_in, w_qkv, w_dw, rescale, w_proj):
    x_in = np.asarray(x_in, dtype=np.float32)
    w_qkv = np.asarray(w_qkv, dtype=np.float32)
    w_dw = np.asarray(w_dw, dtype=np.float32)
    rescale = np.asarray(rescale, dtype=np.float32)
    w_proj = np.asarray(w_proj, dtype=np.float32)

    # x: NHWC -> NCHW, pad 2 halo rows top/bottom
    xT = np.transpose(x_in, (0, 3, 1, 2))                    # [B, C, H, W]
    xpad = np.zeros((B, C, H + 4, W), np.float32)
    xpad[:, :, 2 : H + 2, :] = xT

    # weights in device layouts
    wq_h = w_qkv[:, :, 0, 0]                                 # [768, 256]
    wq_l = np.ascontiguousarray(
        wq_h.T.reshape(CT, 128, 768).transpose(1, 0, 2)
    )                                                        # [128, CT, 768]
    wdw_l = np.ascontiguousarray(
        w_dw[:, 0].reshape(6, 128, 9).transpose(1, 0, 2)
    )                                                        # [128, 6, 9]
    wp_l = np.ascontiguousarray(
        w_proj.transpose(1, 2, 3, 0)                         # [i, 3, 3, o]
        .reshape(C, 9, C)
        .reshape(CT, 128, 9, C)
        .transpose(1, 0, 2, 3)
    )                                                        # [128, CT, 9, 256]
    ident = np.eye(128, dtype=np.float32)
    resc_l = np.empty((128, CT), np.float32)
    r = rescale.reshape(HEADS)
    for g in range(CT):
        resc_l[:, g] = np.repeat(r[4 * g : 4 * g + 4], 32)

    in_maps = []
    for i in range(N_CORES):
        band = np.ascontiguousarray(
            xpad[:, :, 32 * i : 32 * i + XB, :]
        ).reshape(B, CT, 128, XB, 256)
        hm = np.ones((128, 2), np.float32)
        if i == 0:
            hm[:, 0] = 0.0
        if i == N_CORES - 1:
            hm[:, 1] = 0.0
        in_maps.append(
            {
                "x": band,
                "wq": wq_l,
                "wdw": wdw_l,
                "wp": wp_l,
                "ident": ident,
                "resc": resc_l,
                "hmask": hm,
            }
        )

    nc = _get_program()
    res = bass_utils.run_bass_kernel_spmd(
        nc, in_maps, core_ids=list(range(N_CORES))
    )

    v_out = np.empty((B, C, H, W), np.float32)
    outc = np.empty((B, C, H, W), np.float32)
    for i in range(N_CORES):
        vb = res.results[i]["vband"]                 # [B, CT, 128, VB, 256]
        oc = res.results[i]["outc"]                  # [B, CT, 128, ROWS, 256]
        v_out[:, :, 32 * i : 32 * i + 32, :] = vb[:, :, :, 1:33, :].reshape(
            B, C, 32, 256
        )
        outc[:, :, 32 * i : 32 * i + 32, :] = oc.reshape(B, C, 32, 256)

    out_c = np.ascontiguousarray(np.transpose(outc, (0, 2, 3, 1)))
    return (out_c, v_out)
